# revision 7
# baseline (speedup 1.0000x reference)
"""Trainium2 Bass kernel for GNN message passing (nn_BDLModule_34488587387542).

Computation (N=100000 nodes, E=1600000 edges, DIM=128):
    deg  = out-degree(src);  a = rsqrt(deg)
    h0   = per-node block rotation of x (8 bundles of 4x4)
    h2   = S S h0,  S = diag(a) A^T diag(a)   (2 propagation steps)
    h3   = inverse rotation of h2
    out  = GELU_exact(h3 @ w1.T + b1) @ w2.T + b2

Sharding: nodes partitioned contiguously across 8 cores (12500 each). Edges
bucketed by owning dst shard; per core, grouped by (dst tile of 128, src
chunk of 25088 replica rows) so gathers use int16 indices. The propagation
step is: dma_gather rows from the replicated node table (SWDGE on Pool —
the only work left on Pool), build one-hot dst matrices on DVE in one
batched is_equal per dst tile (tile-major packed dst stream), accumulate
with PE matmuls into PSUM. AllGather replicates the node table between
steps. The separable norm coefs a[src]*a[dst] are folded into the stored
node tables, never per-edge. Rotations run batched per super-tile on DVE
via the affine (q b)-merged view; FFN matmuls are batched across 4-tile
chunks of nodes. All DVE-path data is fp16 (fp32 accumulate in PSUM).
"""
import os
import sys

sys.path.append("/opt/trn_rl_repo")

import numpy as np

N_NODES = 100000
N_EDGES = 1600000
DIM = 128
HID = 256
N_CORES = 8
NSH = 12500                 # nodes per shard
NSHP = 12544                # padded shard rows (98 * 128)
NT = NSHP // 128            # dst tiles per core = 98
NREP = NSHP * N_CORES       # replica table rows = 100352
CHUNK = NREP // 4           # gather chunk rows = 25088 (int16-addressable)
N_CHUNKS = 4
PAD_DST = 1000.0            # dst_local sentinel for padding edges
TS = 7                      # dst tiles per super-tile (gathers merged per
NS = NT // TS               # (super-tile, chunk) to amortize SWDGE overhead)

# module globals: last run state (test.py reuses these for timing)
LAST_RESULTS = None
LAST_NC = None
LAST_IN_MAPS = None


# ----------------------------------------------------------------- host prep

def _wrap_idx(idx_flat: np.ndarray) -> np.ndarray:
    """[n] -> [128, n/16] int16 wrapped+replicated dma_gather index layout."""
    w = idx_flat.reshape(-1, 16).T.astype(np.int16)
    return np.tile(w, (8, 1))


def preprocess(x, node_rep, src, dst, w1, b1, w2, b2):
    """Build per-core input maps + the static SPMD edge-group structure."""
    deg = np.bincount(src, minlength=N_NODES).astype(np.float64)
    a64 = 1.0 / np.sqrt(deg)
    a = a64.astype(np.float32)
    a2 = (1.0 / deg).astype(np.float32)

    # global node id -> replica-table row
    def rrow(u):
        return (u // NSH) * NSHP + (u % NSH)

    src_rrow = rrow(src)
    dst_core = dst // NSH

    # Edge stream packed per (super-tile s, chunk k): the 7 tiles' edges
    # concatenated (tile order), padded to a multiple of 128 shared across
    # cores. Groups of 128 may straddle tile boundaries; a straddling group
    # is visited by both tiles with complementary PAD masks in their dst
    # streams, so the gather stream carries ~4% padding instead of ~25%.
    per_core = []
    cnt_tuk = np.zeros((N_CORES, NS, N_CHUNKS, TS), np.int64)
    for c in range(N_CORES):
        m = dst_core == c
        dl = (dst[m] - c * NSH).astype(np.int64)      # local dst
        sr = src_rrow[m]
        tile_id = dl // 128
        chunk_id = sr // CHUNK
        key = ((tile_id // TS) * N_CHUNKS + chunk_id) * TS + tile_id % TS
        order = np.argsort(key, kind="stable")
        per_core.append((dl[order], sr[order], key[order]))
        np.add.at(cnt_tuk, (c, tile_id // TS, chunk_id, tile_id % TS), 1)

    cnt_sk = cnt_tuk.sum(axis=3)                      # [C, NS, K]
    Gsk = np.ceil(cnt_sk.max(axis=0) / 128.0).astype(np.int64)   # [NS, K]
    gsk_start = np.concatenate(
        [[0], np.cumsum(Gsk.reshape(-1))]).reshape(-1)[:-1].reshape(NS,
                                                                    N_CHUNKS)
    total_groups = int(Gsk.sum())
    total_edges_padded = total_groups * 128

    # per-(core, s, k, u): start offset of tile u's edges in the (s,k) stream
    start_u = np.concatenate(
        [np.zeros((N_CORES, NS, N_CHUNKS, 1), np.int64),
         np.cumsum(cnt_tuk, axis=3)], axis=3)          # [C, NS, K, TS+1]
    # shared group ranges per (tile, chunk): groups the tile may touch on
    # any core
    lo = np.zeros((NT, N_CHUNKS), np.int64)
    hi = np.zeros((NT, N_CHUNKS), np.int64)
    for su in range(NS):
        for k in range(N_CHUNKS):
            for u in range(TS):
                t = su * TS + u
                s0 = start_u[:, su, k, u].min() // 128
                e1 = start_u[:, su, k, u + 1].max()
                e1 = -(-e1 // 128)
                if e1 <= s0:
                    s0 = e1 = 0
                lo[t, k] = gsk_start[su, k] + s0
                hi[t, k] = gsk_start[su, k] + e1
    gt_per_tile = (hi - lo).sum(axis=1)
    tm_off = np.concatenate([[0], np.cumsum(gt_per_tile)])
    total_tm = int(tm_off[-1])
    gt_max = int(gt_per_tile.max())
    structure = {"Gsk": Gsk, "gsk_start": gsk_start, "lo": lo, "hi": hi,
                 "tm_off": tm_off, "total_tm": total_tm}

    # iota3[p, c, g] = c: one-hot comparand with the group axis innermost so
    # every operand of the batched is_equal is stride-1 (DVE 2x mode)
    iota3 = np.ascontiguousarray(np.broadcast_to(
        np.arange(128, dtype=np.float16)[None, :, None], (128, 128, gt_max)))
    iota = np.tile(np.arange(128, dtype=np.float16), (128, 1))
    ident = np.eye(128, dtype=np.float16)
    w1t = np.ascontiguousarray(w1.T).astype(np.float16)        # [DIM, HID]
    b1h = np.ascontiguousarray(b1.reshape(2, 128).T)           # [128, 2]
    w2t2 = np.ascontiguousarray(
        w2.T.reshape(2, 128, DIM).transpose(1, 0, 2)).astype(np.float16)
    b2c = np.ascontiguousarray(b2.reshape(128, 1))

    in_maps = []
    for c in range(N_CORES):
        dl, sr, key = per_core[c]
        # slot position of each edge: groups packed per (s,k) bucket
        bucket_starts = (gsk_start.reshape(-1) * 128)[
            np.arange(NS * N_CHUNKS).reshape(NS, N_CHUNKS).repeat(TS).reshape(
                NS, N_CHUNKS, TS).reshape(-1)]        # [n_buckets] by key
        n_buckets = NS * N_CHUNKS * TS
        bcnt = np.bincount(key, minlength=n_buckets)
        within = np.arange(dl.shape[0]) - np.concatenate(
            [[0], np.cumsum(bcnt)])[key]
        # offset of tile-u block inside its (s,k) stream
        su_of = key // (N_CHUNKS * TS)
        k_of = (key // TS) % N_CHUNKS
        u_of = key % TS
        pos = (bucket_starts[key] + start_u[c, su_of, k_of, u_of] + within)
        idx_pad = np.zeros(total_edges_padded, np.int64)
        dst_pad = np.full(total_edges_padded, PAD_DST, np.float32)
        tile_pad = np.full(total_edges_padded, -1, np.int64)
        idx_pad[pos] = sr % CHUNK
        dst_pad[pos] = (dl % 128).astype(np.float32)
        tile_pad[pos] = dl // 128

        # tile-major masked dst stream: for tile t, chunk k, groups
        # [lo, hi): dst row where the slot's edge belongs to t, else PAD
        dst_tm = np.full((total_tm, 128), PAD_DST, np.float32)
        col = 0
        dg = dst_pad.reshape(total_groups, 128)
        tg = tile_pad.reshape(total_groups, 128)
        for t in range(NT):
            for k in range(N_CHUNKS):
                for g in range(int(lo[t, k]), int(hi[t, k])):
                    mask = tg[g] == t
                    dst_tm[col, mask] = dg[g, mask]
                    col += 1
        assert col == total_tm

        rows = slice(c * NSH, (c + 1) * NSH)
        x_sh = np.zeros((NSHP, DIM), np.float16)
        x_sh[:NSH] = x[rows].astype(np.float16)
        rep_sh = np.zeros((NSHP, DIM), np.float16)
        rep_sh[:NSH] = node_rep[rows].reshape(NSH, DIM).astype(np.float16)
        a_pad = np.zeros(NSHP, np.float32)
        a_pad[:NSH] = a[rows]
        a2_pad = np.zeros(NSHP, np.float32)
        a2_pad[:NSH] = a2[rows]

        in_maps.append({
            "x_sh": x_sh,
            "rep_sh": rep_sh,
            "idx_all": _wrap_idx(idx_pad),                       # [128, tg*8]
            "dst_all": np.ascontiguousarray(
                dst_tm.T.astype(np.float16)),                    # [128, ttm]
            "a_col": np.ascontiguousarray(
                a_pad.reshape(NT, 128).T),                       # [128, NT]
            "a2_col": np.ascontiguousarray(
                a2_pad.reshape(NT, 128).T),                      # [128, NT]
            "iota": iota,
            "iota3": iota3,
            "ident": ident,
            "w1t": w1t,
            "b1h": b1h,
            "w2t2": w2t2,
            "b2c": b2c,
        })
    return in_maps, structure, total_groups


# -------------------------------------------------------------- device build

def build_nc(structure, total_groups, single_core_timing=False, ablate=(),
             n_queues=4, gather_split=1):
    import concourse.bacc as bacc
    import concourse.mybir as mybir
    import concourse.tile as tile

    f32 = mybir.dt.float32
    f16 = mybir.dt.float16
    nc = bacc.Bacc("TRN2", target_bir_lowering=False, debug=False,
                   num_devices=1 if single_core_timing else N_CORES,
                   num_swdge_queues=n_queues)

    x_sh = nc.dram_tensor("x_sh", [NSHP, DIM], f16, kind="ExternalInput")
    rep_sh = nc.dram_tensor("rep_sh", [NSHP, DIM], f16, kind="ExternalInput")
    idx_all = nc.dram_tensor("idx_all", [128, total_groups * 8],
                             mybir.dt.int16, kind="ExternalInput")
    total_tm = structure["total_tm"]
    dst_all = nc.dram_tensor("dst_all", [128, total_tm], f16,
                             kind="ExternalInput")
    a_col = nc.dram_tensor("a_col", [128, NT], f32, kind="ExternalInput")
    a2_col = nc.dram_tensor("a2_col", [128, NT], f32, kind="ExternalInput")
    iota = nc.dram_tensor("iota", [128, 128], f16, kind="ExternalInput")
    ident = nc.dram_tensor("ident", [128, 128], f16, kind="ExternalInput")
    w1t = nc.dram_tensor("w1t", [DIM, HID], f16, kind="ExternalInput")
    b1h = nc.dram_tensor("b1h", [128, 2], f32, kind="ExternalInput")
    w2t2 = nc.dram_tensor("w2t2", [128, 2, DIM], f16, kind="ExternalInput")
    b2c = nc.dram_tensor("b2c", [128, 1], f32, kind="ExternalInput")
    out_t = nc.dram_tensor("out_t", [DIM, NSHP], f32, kind="ExternalOutput")

    Gsk = structure["Gsk"]
    gsk_start = structure["gsk_start"]
    lo = structure["lo"]
    hi = structure["hi"]
    tm_off = structure["tm_off"]
    gsk_max = int(Gsk.max())
    gt_max = int((hi - lo).sum(axis=1).max())
    iota3 = nc.dram_tensor("iota3", [128, 128, gt_max], f16,
                           kind="ExternalInput")
    with tile.TileContext(nc) as tc:
        with (
            tc.tile_pool(name="const", bufs=1) as cp,
            tc.tile_pool(name="io", bufs=2) as iop,
            tc.tile_pool(name="rotp", bufs=2) as rotp,
            tc.tile_pool(name="gath", bufs=8) as gp,
            tc.tile_pool(name="m2", bufs=4) as m2p,
            tc.tile_pool(name="outp", bufs=2) as op,
            tc.tile_pool(name="psA", bufs=3, space="PSUM") as ppA,
            tc.tile_pool(name="psB", bufs=1, space="PSUM") as ppB,
            tc.tile_pool(name="dram", bufs=1, space="DRAM") as dp,
        ):
            # ---- constants into SBUF
            iota_sb = cp.tile([128, 128], f16)
            nc.sync.dma_start(iota_sb[:], iota[:])
            iota3_sb = cp.tile([128, 128, gt_max], f16)
            nc.sync.dma_start(iota3_sb[:], iota3[:])
            id_sb = cp.tile([128, 128], f16)
            nc.sync.dma_start(id_sb[:], ident[:])
            idx_sb = cp.tile([128, total_groups * 8], mybir.dt.int16)
            nc.sync.dma_start(idx_sb[:], idx_all[:])
            dst_sb = cp.tile([128, total_tm], f16)
            nc.sync.dma_start(dst_sb[:], dst_all[:])
            a_sb = cp.tile([128, NT], f32)
            nc.sync.dma_start(a_sb[:], a_col[:])
            a2_sb = cp.tile([128, NT], f32)
            nc.sync.dma_start(a2_sb[:], a2_col[:])
            w1t_sb = cp.tile([DIM, HID], f16)
            nc.sync.dma_start(w1t_sb[:], w1t[:])
            b1h_sb = cp.tile([128, 2], f32)
            nc.sync.dma_start(b1h_sb[:], b1h[:])
            w2t2_sb = cp.tile([128, 2, DIM], f16)
            nc.sync.dma_start(w2t2_sb[:], w2t2[:])
            b2c_sb = cp.tile([128, 1], f32)
            nc.sync.dma_start(b2c_sb[:], b2c[:])

            rep_space = "Local" if single_core_timing else "Shared"
            g0_sh = dp.tile([NSHP, DIM], f16)
            g0_rep = dp.tile([NREP, DIM], f16, addr_space=rep_space)
            g1_sh = dp.tile([NSHP, DIM], f16)
            g1_rep = dp.tile([NREP, DIM], f16, addr_space=rep_space)

            def allgather(sh, rep):
                if single_core_timing:
                    # timing stand-in: local copy keeps the data dependency
                    nc.sync.dma_start(rep[0:NSHP, :], sh[:])
                else:
                    nc.gpsimd.collective_compute(
                        "AllGather", mybir.AluOpType.bypass,
                        ins=[sh.opt()], outs=[rep.opt()],
                        replica_groups=[list(range(N_CORES))],
                    )

            def rotation_q(x_ap, rep_ap, dest, dest_tag, transposed, q=TS):
                """dest[:, q, :] = per-node block rotation, batched over q
                tiles. The (q, bundle) axes merge into one affine dim m
                (q-stride 128 = 8 x bundle-stride 16)."""
                m = q * 8
                x4 = x_ap.rearrange("p q (b d e) -> p (q b) d e", b=8, d=4,
                                    e=4)
                r4 = rep_ap.rearrange("p q (b c d) -> p (q b) c d", b=8, c=4,
                                      d=4)
                tmp = rotp.tile([128, q, DIM], f16, tag=dest_tag + "_tmp")
                for d in range(4):
                    if transposed:
                        # out[m,c,e] += rep[m,d,c] * x[m,d,e]
                        a_d = r4[:, :, d, :].unsqueeze(3)
                    else:
                        # out[m,c,e] += rep[m,c,d] * x[m,d,e]
                        a_d = r4[:, :, :, d].unsqueeze(3)
                    a_d = a_d.broadcast_to((128, m, 4, 4))
                    b_d = x4[:, :, d, :].unsqueeze(2).broadcast_to(
                        (128, m, 4, 4))
                    dst4 = (dest if d == 0 else tmp)[:].rearrange(
                        "p q (b c e) -> p (q b) c e", b=8, c=4, e=4)
                    nc.vector.tensor_tensor(dst4, a_d, b_d,
                                            op=mybir.AluOpType.mult)
                    if d > 0:
                        nc.vector.tensor_tensor(dest[:], dest[:], tmp[:],
                                                op=mybir.AluOpType.add)

            def sh_rows(dram, s):
                """[128, TS, DIM] view of a shard's super-tile s rows."""
                return dram[s * TS * 128:(s + 1) * TS * 128, :].rearrange(
                    "(q p) d -> p q d", p=128)

            # ---- phase R1: g0 = rotate(x) * a
            for s in range(NS):
                xp = iop.tile([128, TS, DIM], f16, tag="xp")
                nc.sync.dma_start(xp[:], sh_rows(x_sh, s))
                rp = iop.tile([128, TS, DIM], f16, tag="rp")
                nc.sync.dma_start(rp[:], sh_rows(rep_sh, s))
                rot = rotp.tile([128, TS, DIM], f16, tag="rot")
                rotation_q(xp[:], rp[:], rot, "rot", transposed=False)
                g0p = op.tile([128, TS, DIM], f16, tag="g0p")
                for u in range(TS):
                    t = s * TS + u
                    nc.scalar.mul(g0p[:, u, :], rot[:, u, :],
                                  a_sb[:, t:t + 1])
                nc.sync.dma_start(sh_rows(g0_sh, s), g0p[:])

            allgather(g0_sh, g0_rep)

            def onehots(t, tag):
                """[128, 128, gt] one-hot stack for dst tile t (one DVE op;
                group axis innermost so all operands are stride-1 -> 2x)."""
                gt = int(tm_off[t + 1] - tm_off[t])
                o0 = int(tm_off[t])
                m2t = m2p.tile([128, 128, gt_max], f16, tag=tag)
                if "onehot" not in ablate:
                    nc.vector.tensor_tensor(
                        m2t[:, :, 0:gt],
                        iota3_sb[:, :, 0:gt],
                        dst_sb[:, o0:o0 + gt].unsqueeze(1).broadcast_to(
                            (128, 128, gt)),
                        op=mybir.AluOpType.is_equal)
                else:  # timing placeholder: tiny op
                    nc.vector.tensor_scalar(
                        m2t[:, 0:1, 0:8], iota_sb[:, 0:8], 0.0,
                        None, op0=mybir.AluOpType.mult)
                return m2t

            def prop_step(g_rep, alloc_cb, emit_cb, flush_cb):
                """One propagation step. One merged gather per (super-tile,
                chunk); per dst tile, a batched one-hot build then one-hot
                matmuls accumulating in PSUM. Outputs are batched per
                super-tile via the callbacks."""
                for su in range(NS):
                    gths = {}
                    for k in range(N_CHUNKS):
                        gc = int(Gsk[su][k])
                        if gc == 0:
                            continue
                        c0 = int(gsk_start[su][k])
                        gth = gp.tile([128, gsk_max, DIM], f16, tag="gth")
                        if "gather" not in ablate:
                            bounds = [gc * i // gather_split
                                      for i in range(gather_split + 1)]
                            for h in range(gather_split):
                                glo, ghi = bounds[h], bounds[h + 1]
                                if glo == ghi:
                                    continue
                                nc.gpsimd.dma_gather(
                                    gth[:, glo:ghi, :],
                                    g_rep[k * CHUNK:(k + 1) * CHUNK, :],
                                    idx_sb[:, (c0 + glo) * 8:(c0 + ghi) * 8],
                                    128 * (ghi - glo), 128 * (ghi - glo),
                                    DIM,
                                    single_packet=False,
                                    queue_num=(gather_split * k + h)
                                    % n_queues,
                                )
                        else:  # timing placeholder: cheap small copy
                            nc.sync.dma_start(gth[:, 0, :], g_rep[0:128, :])
                        gths[k] = gth
                    bt = alloc_cb(su)
                    for u in range(TS):
                        t = su * TS + u
                        m2t = onehots(t, "m2t")
                        acc = ppA.tile([128, DIM], f32, tag="acc")
                        n_mm = int((hi[t] - lo[t]).sum())
                        assert n_mm > 0
                        mm = 0
                        for k in range(N_CHUNKS):
                            base = int(gsk_start[su][k])
                            for g in range(int(lo[t][k]), int(hi[t][k])):
                                if "mm" not in ablate or mm == 0:
                                    nc.tensor.matmul(
                                        acc[:], m2t[:, :, mm],
                                        gths[k][:, g - base, :],
                                        start=(mm == 0),
                                        stop=(mm == n_mm - 1
                                              or "mm" in ablate),
                                    )
                                mm += 1
                        emit_cb(t, u, acc, bt)
                    flush_cb(su, bt)

            # ---- phase P1: g1 = A^T g0, scaled by a^2
            prop_step(
                g0_rep,
                lambda s: op.tile([128, TS, DIM], f16, tag="g1p", name="g1p"),
                lambda t, u, acc, bt: nc.scalar.mul(
                    bt[:, u, :], acc[:], a2_sb[:, t:t + 1]),
                lambda s, bt: nc.sync.dma_start(sh_rows(g1_sh, s), bt[:]),
            )

            allgather(g1_sh, g1_rep)

            # ---- phase P2 + inverse rotation + FFN, fused per super-tile
            state = {}

            def p2_alloc(s):
                rp2 = iop.tile([128, TS, DIM], f16, tag="rp2")
                nc.sync.dma_start(rp2[:], sh_rows(rep_sh, s))
                h2s = rotp.tile([128, TS, DIM], f16, tag="h2s")
                state["rp2"] = rp2
                state["h2s"] = h2s
                return op.tile([128, TS, DIM], f32, tag="op2", name="op2")

            def p2_emit(t, u, acc, bt):
                # PSUM evict + a-scale into the super-tile h2 buffer
                nc.scalar.mul(state["h2s"][:, u, :], acc[:],
                              a_sb[:, t:t + 1])

            def p2_flush(s, bt):
                h2s, rp2 = state["h2s"], state["rp2"]
                h3 = rotp.tile([128, TS, DIM], f16, tag="h3")
                rotation_q(h2s[:], rp2[:], h3, "h3", transposed=True)
                # transpose each tile to [feat, node]
                h3t = rotp.tile([128, TS, 128], f16, tag="h3t")
                for u in range(TS):
                    tp = ppA.tile([128, 128], f16, tag="tp")
                    nc.tensor.transpose(tp[:], h3[:, u, :], id_sb[:])
                    nc.scalar.copy(h3t[:, u, :], tp[:])
                # FFN over node chunks of <=4 tiles (512 cols)
                act = rotp.tile([128, 2, TS, 128], f16, tag="act")
                for c0, cw in ((0, 4), (4, 3)):
                    w = cw * 128
                    h3c = h3t[:, c0:c0 + cw, :].rearrange("p q d -> p (q d)")
                    for h in range(2):
                        ps1 = ppB.tile([128, 512], f32, tag="ps1")
                        nc.tensor.matmul(ps1[:, 0:w],
                                         w1t_sb[:, h * 128:(h + 1) * 128],
                                         h3c, start=True, stop=True)
                        nc.scalar.activation(
                            act[:, h, c0:c0 + cw, :].rearrange(
                                "p q d -> p (q d)"),
                            ps1[:, 0:w],
                            mybir.ActivationFunctionType.Gelu,
                            bias=b1h_sb[:, h:h + 1])
                    ps2 = ppB.tile([128, 512], f32, tag="ps2")
                    for h in range(2):
                        nc.tensor.matmul(
                            ps2[:, 0:w], w2t2_sb[:, h, :],
                            act[:, h, c0:c0 + cw, :].rearrange(
                                "p q d -> p (q d)"),
                            start=(h == 0), stop=(h == 1))
                    nc.scalar.activation(
                        bt[:, c0:c0 + cw, :].rearrange("p q d -> p (q d)"),
                        ps2[:, 0:w],
                        mybir.ActivationFunctionType.Identity,
                        bias=b2c_sb[:])
                nc.sync.dma_start(
                    out_t[:, s * TS * 128:(s + 1) * TS * 128],
                    bt[:].rearrange("p q d -> p (q d)"))

            prop_step(g1_rep, p2_alloc, p2_emit, p2_flush)

    nc.compile()
    return nc


# -------------------------------------------------------------------- runner

def kernel(x, node_rep, src, dst, w1, b1, w2, b2):
    global LAST_RESULTS, LAST_NC, LAST_IN_MAPS
    from concourse import bass_utils

    x = np.asarray(x, np.float32)
    node_rep = np.asarray(node_rep, np.float32)
    src = np.asarray(src, np.int64)
    dst = np.asarray(dst, np.int64)
    w1 = np.asarray(w1, np.float32)
    b1 = np.asarray(b1, np.float32)
    w2 = np.asarray(w2, np.float32)
    b2 = np.asarray(b2, np.float32)

    in_maps, structure, total_groups = preprocess(
        x, node_rep, src, dst, w1, b1, w2, b2)
    nc = build_nc(structure, total_groups)
    res = bass_utils.run_bass_kernel_spmd(
        nc, in_maps, core_ids=list(range(N_CORES)),
        trace=bool(os.environ.get("BASS_TRACE")),
    )
    LAST_RESULTS = res
    LAST_NC = nc
    LAST_IN_MAPS = in_maps
    out = np.concatenate(
        [res.results[c]["out_t"].T[:NSH] for c in range(N_CORES)], axis=0)
    return np.ascontiguousarray(out)


# revision 9
# speedup vs baseline: 2.8766x; 2.8766x over previous
"""Trainium2 Bass kernel for GNN message passing (nn_BDLModule_34488587387542).

Computation (N=100000 nodes, E=1600000 edges, DIM=128):
    deg  = out-degree(src);  a = rsqrt(deg)
    h0   = per-node block rotation of x (8 bundles of 4x4)
    h2   = S S h0,  S = diag(a) A^T diag(a)   (2 propagation steps)
    h3   = inverse rotation of h2
    out  = GELU_exact(h3 @ w1.T + b1) @ w2.T + b2

Sharding: nodes partitioned contiguously across 8 cores (12500 each). Edges
bucketed by owning dst shard; per core, grouped by (dst tile of 128, src
chunk of 25088 replica rows) so gathers use int16 indices. The propagation
step is: dma_gather rows from the replicated node table (SWDGE on Pool —
the only work left on Pool), build one-hot dst matrices on DVE in one
batched is_equal per dst tile (tile-major packed dst stream), accumulate
with PE matmuls into PSUM. AllGather replicates the node table between
steps. The separable norm coefs a[src]*a[dst] are folded into the stored
node tables, never per-edge. Rotations run batched per super-tile on DVE
via the affine (q b)-merged view; FFN matmuls are batched across 4-tile
chunks of nodes. All DVE-path data is fp16 (fp32 accumulate in PSUM).
"""
import os
import sys

sys.path.append("/opt/trn_rl_repo")

import numpy as np

N_NODES = 100000
N_EDGES = 1600000
DIM = 128
HID = 256
N_CORES = 8
NSH = 12500                 # nodes per shard
NSHP = 12544                # padded shard rows (98 * 128)
NT = NSHP // 128            # dst tiles per core = 98
NREP = NSHP * N_CORES       # replica table rows = 100352
CHUNK = NREP // 4           # gather chunk rows = 25088 (int16-addressable)
N_CHUNKS = 4
PAD_DST = 1000.0            # dst_local sentinel for padding edges
TS = 7                      # dst tiles per super-tile (gathers merged per
NS = NT // TS               # (super-tile, chunk) to amortize SWDGE overhead)

# module globals: last run state (test.py reuses these for timing)
LAST_RESULTS = None
LAST_NC = None
LAST_IN_MAPS = None


# ----------------------------------------------------------------- host prep

def _wrap_idx(idx_flat: np.ndarray) -> np.ndarray:
    """[n] -> [128, n/16] int16 wrapped+replicated dma_gather index layout."""
    w = idx_flat.reshape(-1, 16).T.astype(np.int16)
    return np.tile(w, (8, 1))


def preprocess(x, node_rep, src, dst, w1, b1, w2, b2):
    """Build per-core input maps + the static SPMD edge-group structure."""
    deg = np.bincount(src, minlength=N_NODES).astype(np.float64)
    a64 = 1.0 / np.sqrt(deg)
    a = a64.astype(np.float32)
    a2 = (1.0 / deg).astype(np.float32)

    # global node id -> replica-table row. Layout is half-shard major:
    # half h (rows [h*6272, ...) of each shard) occupies replica rows
    # [h*50176 + shard*6272, ...+6272), so chunks 0-1 depend only on the
    # first-half AllGather and chunks 2-3 on the second.
    HALF = NSHP // 2

    def rrow(u):
        c = u // NSH
        r = u % NSH
        h = r // HALF
        return h * (N_CORES * HALF) + c * HALF + (r - h * HALF)

    src_rrow = rrow(src)
    dst_core = dst // NSH

    # Edge stream packed per (super-tile s, chunk k): the 7 tiles' edges
    # concatenated (tile order), padded to a multiple of 128 shared across
    # cores. Groups of 128 may straddle tile boundaries; a straddling group
    # is visited by both tiles with complementary PAD masks in their dst
    # streams, so the gather stream carries ~4% padding instead of ~25%.
    per_core = []
    cnt_tuk = np.zeros((N_CORES, NS, N_CHUNKS, TS), np.int64)
    for c in range(N_CORES):
        m = dst_core == c
        dl = (dst[m] - c * NSH).astype(np.int64)      # local dst
        sr = src_rrow[m]
        tile_id = dl // 128
        chunk_id = sr // CHUNK
        key = ((tile_id // TS) * N_CHUNKS + chunk_id) * TS + tile_id % TS
        order = np.argsort(key, kind="stable")
        per_core.append((dl[order], sr[order], key[order]))
        np.add.at(cnt_tuk, (c, tile_id // TS, chunk_id, tile_id % TS), 1)

    cnt_sk = cnt_tuk.sum(axis=3)                      # [C, NS, K]
    Gsk = np.ceil(cnt_sk.max(axis=0) / 128.0).astype(np.int64)   # [NS, K]
    gsk_start = np.concatenate(
        [[0], np.cumsum(Gsk.reshape(-1))]).reshape(-1)[:-1].reshape(NS,
                                                                    N_CHUNKS)
    total_groups = int(Gsk.sum())
    total_edges_padded = total_groups * 128

    # per-(core, s, k, u): start offset of tile u's edges in the (s,k) stream
    start_u = np.concatenate(
        [np.zeros((N_CORES, NS, N_CHUNKS, 1), np.int64),
         np.cumsum(cnt_tuk, axis=3)], axis=3)          # [C, NS, K, TS+1]
    # shared group ranges per (tile, chunk): groups the tile may touch on
    # any core
    lo = np.zeros((NT, N_CHUNKS), np.int64)
    hi = np.zeros((NT, N_CHUNKS), np.int64)
    for su in range(NS):
        for k in range(N_CHUNKS):
            for u in range(TS):
                t = su * TS + u
                s0 = start_u[:, su, k, u].min() // 128
                e1 = start_u[:, su, k, u + 1].max()
                e1 = -(-e1 // 128)
                if e1 <= s0:
                    s0 = e1 = 0
                lo[t, k] = gsk_start[su, k] + s0
                hi[t, k] = gsk_start[su, k] + e1
    gt_per_tile = (hi - lo).sum(axis=1)
    tm_off = np.concatenate([[0], np.cumsum(gt_per_tile)])
    total_tm = int(tm_off[-1])
    gt_max = int(gt_per_tile.max())
    structure = {"Gsk": Gsk, "gsk_start": gsk_start, "lo": lo, "hi": hi,
                 "tm_off": tm_off, "total_tm": total_tm}

    # iota3[p, c, g] = c: one-hot comparand with the group axis innermost so
    # every operand of the batched is_equal is stride-1 (DVE 2x mode)
    iota3 = np.ascontiguousarray(np.broadcast_to(
        np.arange(128, dtype=np.float16)[None, :, None], (128, 128, gt_max)))
    iota = np.tile(np.arange(128, dtype=np.float16), (128, 1))
    ident = np.eye(128, dtype=np.float16)
    w1t = np.ascontiguousarray(w1.T).astype(np.float16)        # [DIM, HID]
    b1h = np.ascontiguousarray(b1.reshape(2, 128).T)           # [128, 2]
    w2t2 = np.ascontiguousarray(
        w2.T.reshape(2, 128, DIM).transpose(1, 0, 2)).astype(np.float16)
    b2c = np.ascontiguousarray(b2.reshape(128, 1))

    in_maps = []
    for c in range(N_CORES):
        dl, sr, key = per_core[c]
        # slot position of each edge: groups packed per (s,k) bucket
        bucket_starts = (gsk_start.reshape(-1) * 128)[
            np.arange(NS * N_CHUNKS).reshape(NS, N_CHUNKS).repeat(TS).reshape(
                NS, N_CHUNKS, TS).reshape(-1)]        # [n_buckets] by key
        n_buckets = NS * N_CHUNKS * TS
        bcnt = np.bincount(key, minlength=n_buckets)
        within = np.arange(dl.shape[0]) - np.concatenate(
            [[0], np.cumsum(bcnt)])[key]
        # offset of tile-u block inside its (s,k) stream
        su_of = key // (N_CHUNKS * TS)
        k_of = (key // TS) % N_CHUNKS
        u_of = key % TS
        pos = (bucket_starts[key] + start_u[c, su_of, k_of, u_of] + within)
        idx_pad = np.zeros(total_edges_padded, np.int64)
        dst_pad = np.full(total_edges_padded, PAD_DST, np.float32)
        tile_pad = np.full(total_edges_padded, -1, np.int64)
        idx_pad[pos] = sr % CHUNK
        dst_pad[pos] = (dl % 128).astype(np.float32)
        tile_pad[pos] = dl // 128

        # tile-major masked dst stream: for tile t, chunk k, groups
        # [lo, hi): dst row where the slot's edge belongs to t, else PAD
        dst_tm = np.full((total_tm, 128), PAD_DST, np.float32)
        col = 0
        dg = dst_pad.reshape(total_groups, 128)
        tg = tile_pad.reshape(total_groups, 128)
        for t in range(NT):
            for k in range(N_CHUNKS):
                for g in range(int(lo[t, k]), int(hi[t, k])):
                    mask = tg[g] == t
                    dst_tm[col, mask] = dg[g, mask]
                    col += 1
        assert col == total_tm

        rows = slice(c * NSH, (c + 1) * NSH)
        x_sh = np.zeros((NSHP, DIM), np.float16)
        x_sh[:NSH] = x[rows].astype(np.float16)
        rep_sh = np.zeros((NSHP, DIM), np.float16)
        rep_sh[:NSH] = node_rep[rows].reshape(NSH, DIM).astype(np.float16)
        a_pad = np.zeros(NSHP, np.float32)
        a_pad[:NSH] = a[rows]
        a2_pad = np.zeros(NSHP, np.float32)
        a2_pad[:NSH] = a2[rows]

        in_maps.append({
            "x_sh": x_sh,
            "rep_sh": rep_sh,
            "idx_all": _wrap_idx(idx_pad),                       # [128, tg*8]
            "dst_all": np.ascontiguousarray(
                dst_tm.T.astype(np.float16)),                    # [128, ttm]
            "a_col": np.ascontiguousarray(
                a_pad.reshape(NT, 128).T),                       # [128, NT]
            "a2_col": np.ascontiguousarray(
                a2_pad.reshape(NT, 128).T),                      # [128, NT]
            "iota": iota,
            "iota3": iota3,
            "ident": ident,
            "w1t": w1t,
            "b1h": b1h,
            "w2t2": w2t2,
            "b2c": b2c,
        })
    return in_maps, structure, total_groups


# -------------------------------------------------------------- device build

def build_nc(structure, total_groups, single_core_timing=False, ablate=(),
             n_queues=4, gather_split=1):
    import concourse.bacc as bacc
    import concourse.mybir as mybir
    import concourse.tile as tile

    f32 = mybir.dt.float32
    f16 = mybir.dt.float16
    nc = bacc.Bacc("TRN2", target_bir_lowering=False, debug=False,
                   num_devices=1 if single_core_timing else N_CORES,
                   num_swdge_queues=n_queues)

    x_sh = nc.dram_tensor("x_sh", [NSHP, DIM], f16, kind="ExternalInput")
    rep_sh = nc.dram_tensor("rep_sh", [NSHP, DIM], f16, kind="ExternalInput")
    idx_all = nc.dram_tensor("idx_all", [128, total_groups * 8],
                             mybir.dt.int16, kind="ExternalInput")
    total_tm = structure["total_tm"]
    dst_all = nc.dram_tensor("dst_all", [128, total_tm], f16,
                             kind="ExternalInput")
    a_col = nc.dram_tensor("a_col", [128, NT], f32, kind="ExternalInput")
    a2_col = nc.dram_tensor("a2_col", [128, NT], f32, kind="ExternalInput")
    iota = nc.dram_tensor("iota", [128, 128], f16, kind="ExternalInput")
    ident = nc.dram_tensor("ident", [128, 128], f16, kind="ExternalInput")
    w1t = nc.dram_tensor("w1t", [DIM, HID], f16, kind="ExternalInput")
    b1h = nc.dram_tensor("b1h", [128, 2], f32, kind="ExternalInput")
    w2t2 = nc.dram_tensor("w2t2", [128, 2, DIM], f16, kind="ExternalInput")
    b2c = nc.dram_tensor("b2c", [128, 1], f32, kind="ExternalInput")
    out_t = nc.dram_tensor("out_t", [DIM, NSHP], f32, kind="ExternalOutput")

    Gsk = structure["Gsk"]
    gsk_start = structure["gsk_start"]
    lo = structure["lo"]
    hi = structure["hi"]
    tm_off = structure["tm_off"]
    gsk_max = int(Gsk.max())
    gt_max = int((hi - lo).sum(axis=1).max())
    iota3 = nc.dram_tensor("iota3", [128, 128, gt_max], f16,
                           kind="ExternalInput")
    with tile.TileContext(nc) as tc:
        with (
            tc.tile_pool(name="const", bufs=1) as cp,
            tc.tile_pool(name="io", bufs=2) as iop,
            tc.tile_pool(name="rotp", bufs=2) as rotp,
            tc.tile_pool(name="gath", bufs=8) as gp,
            tc.tile_pool(name="m2", bufs=4) as m2p,
            tc.tile_pool(name="outp", bufs=2) as op,
            tc.tile_pool(name="psA", bufs=3, space="PSUM") as ppA,
            tc.tile_pool(name="psB", bufs=1, space="PSUM") as ppB,
            tc.tile_pool(name="dram", bufs=1, space="DRAM") as dp,
        ):
            # ---- constants into SBUF
            iota_sb = cp.tile([128, 128], f16)
            nc.sync.dma_start(iota_sb[:], iota[:])
            iota3_sb = cp.tile([128, 128, gt_max], f16)
            nc.sync.dma_start(iota3_sb[:], iota3[:])
            id_sb = cp.tile([128, 128], f16)
            nc.sync.dma_start(id_sb[:], ident[:])
            idx_sb = cp.tile([128, total_groups * 8], mybir.dt.int16)
            nc.sync.dma_start(idx_sb[:], idx_all[:])
            dst_sb = cp.tile([128, total_tm], f16)
            nc.sync.dma_start(dst_sb[:], dst_all[:])
            a_sb = cp.tile([128, NT], f32)
            nc.sync.dma_start(a_sb[:], a_col[:])
            a2_sb = cp.tile([128, NT], f32)
            nc.sync.dma_start(a2_sb[:], a2_col[:])
            w1t_sb = cp.tile([DIM, HID], f16)
            nc.sync.dma_start(w1t_sb[:], w1t[:])
            b1h_sb = cp.tile([128, 2], f32)
            nc.sync.dma_start(b1h_sb[:], b1h[:])
            w2t2_sb = cp.tile([128, 2, DIM], f16)
            nc.sync.dma_start(w2t2_sb[:], w2t2[:])
            b2c_sb = cp.tile([128, 1], f32)
            nc.sync.dma_start(b2c_sb[:], b2c[:])

            rep_space = "Local" if single_core_timing else "Shared"
            HALF = NSHP // 2
            NSH2 = NS // 2
            g0_sh = [dp.tile([HALF, DIM], f16, name=f"g0sh{h}")
                     for h in range(2)]
            g0_rep = [dp.tile([HALF * N_CORES, DIM], f16,
                              addr_space=rep_space, name=f"g0rep{h}")
                      for h in range(2)]
            g1_sh = [dp.tile([HALF, DIM], f16, name=f"g1sh{h}")
                     for h in range(2)]
            g1_rep = [dp.tile([HALF * N_CORES, DIM], f16,
                              addr_space=rep_space, name=f"g1rep{h}")
                      for h in range(2)]

            def allgather(sh, rep):
                # one half-shard AllGather; fires as soon as its half is
                # written, overlapping the other half's compute
                if single_core_timing:
                    # timing stand-in: local copy keeps the data dependency
                    nc.sync.dma_start(rep[0:HALF, :], sh[:])
                else:
                    nc.gpsimd.collective_compute(
                        "AllGather", mybir.AluOpType.bypass,
                        ins=[sh.opt()], outs=[rep.opt()],
                        replica_groups=[list(range(N_CORES))],
                    )

            def rotation_q(x_ap, rep_ap, dest, dest_tag, transposed, q=TS):
                """dest[:, q, :] = per-node block rotation, batched over q
                tiles. The (q, bundle) axes merge into one affine dim m
                (q-stride 128 = 8 x bundle-stride 16)."""
                m = q * 8
                x4 = x_ap.rearrange("p q (b d e) -> p (q b) d e", b=8, d=4,
                                    e=4)
                r4 = rep_ap.rearrange("p q (b c d) -> p (q b) c d", b=8, c=4,
                                      d=4)
                tmp = rotp.tile([128, q, DIM], f16, tag=dest_tag + "_tmp")
                for d in range(4):
                    if transposed:
                        # out[m,c,e] += rep[m,d,c] * x[m,d,e]
                        a_d = r4[:, :, d, :].unsqueeze(3)
                    else:
                        # out[m,c,e] += rep[m,c,d] * x[m,d,e]
                        a_d = r4[:, :, :, d].unsqueeze(3)
                    a_d = a_d.broadcast_to((128, m, 4, 4))
                    b_d = x4[:, :, d, :].unsqueeze(2).broadcast_to(
                        (128, m, 4, 4))
                    dst4 = (dest if d == 0 else tmp)[:].rearrange(
                        "p q (b c e) -> p (q b) c e", b=8, c=4, e=4)
                    nc.vector.tensor_tensor(dst4, a_d, b_d,
                                            op=mybir.AluOpType.mult)
                    if d > 0:
                        nc.vector.tensor_tensor(dest[:], dest[:], tmp[:],
                                                op=mybir.AluOpType.add)

            def sh_rows(dram, s):
                """[128, TS, DIM] view of a shard's super-tile s rows."""
                return dram[s * TS * 128:(s + 1) * TS * 128, :].rearrange(
                    "(q p) d -> p q d", p=128)

            # ---- phase R1: g0 = rotate(x) * a
            for s in range(NS):
                xp = iop.tile([128, TS, DIM], f16, tag="xp")
                nc.sync.dma_start(xp[:], sh_rows(x_sh, s))
                rp = iop.tile([128, TS, DIM], f16, tag="rp")
                nc.sync.dma_start(rp[:], sh_rows(rep_sh, s))
                rot = rotp.tile([128, TS, DIM], f16, tag="rot")
                rotation_q(xp[:], rp[:], rot, "rot", transposed=False)
                g0p = op.tile([128, TS, DIM], f16, tag="g0p")
                for u in range(TS):
                    t = s * TS + u
                    nc.scalar.mul(g0p[:, u, :], rot[:, u, :],
                                  a_sb[:, t:t + 1])
                nc.sync.dma_start(sh_rows(g0_sh[s // NSH2], s % NSH2),
                                  g0p[:])
                if s % NSH2 == NSH2 - 1:
                    allgather(g0_sh[s // NSH2], g0_rep[s // NSH2])

            def onehots(t, tag):
                """[128, 128, gt] one-hot stack for dst tile t (one DVE op;
                group axis innermost so all operands are stride-1 -> 2x)."""
                gt = int(tm_off[t + 1] - tm_off[t])
                o0 = int(tm_off[t])
                m2t = m2p.tile([128, 128, gt_max], f16, tag=tag)
                if "onehot" not in ablate:
                    nc.vector.tensor_tensor(
                        m2t[:, :, 0:gt],
                        iota3_sb[:, :, 0:gt],
                        dst_sb[:, o0:o0 + gt].unsqueeze(1).broadcast_to(
                            (128, 128, gt)),
                        op=mybir.AluOpType.is_equal)
                else:  # timing placeholder: tiny op
                    nc.vector.tensor_scalar(
                        m2t[:, 0:1, 0:8], iota_sb[:, 0:8], 0.0,
                        None, op0=mybir.AluOpType.mult)
                return m2t

            def prop_step(g_rep, alloc_cb, emit_cb, flush_cb):
                """One propagation step. One merged gather per (super-tile,
                chunk); per dst tile, a batched one-hot build then one-hot
                matmuls accumulating in PSUM. Outputs are batched per
                super-tile via the callbacks."""
                for su in range(NS):
                    gths = {}
                    for k in range(N_CHUNKS):
                        gc = int(Gsk[su][k])
                        if gc == 0:
                            continue
                        c0 = int(gsk_start[su][k])
                        gth = gp.tile([128, gsk_max, DIM], f16, tag="gth")
                        if "gather" not in ablate:
                            bounds = [gc * i // gather_split
                                      for i in range(gather_split + 1)]
                            for h in range(gather_split):
                                glo, ghi = bounds[h], bounds[h + 1]
                                if glo == ghi:
                                    continue
                                nc.gpsimd.dma_gather(
                                    gth[:, glo:ghi, :],
                                    g_rep[k // 2][
                                        (k % 2) * CHUNK:
                                        (k % 2 + 1) * CHUNK, :],
                                    idx_sb[:, (c0 + glo) * 8:(c0 + ghi) * 8],
                                    128 * (ghi - glo), 128 * (ghi - glo),
                                    DIM,
                                    single_packet=False,
                                    queue_num=(gather_split * k + h)
                                    % n_queues,
                                )
                        else:  # timing placeholder: cheap small copy
                            nc.sync.dma_start(gth[:, 0, :],
                                              g_rep[0][0:128, :])
                        gths[k] = gth
                    bt = alloc_cb(su)
                    for u in range(TS):
                        t = su * TS + u
                        m2t = onehots(t, "m2t")
                        acc = ppA.tile([128, DIM], f32, tag="acc")
                        n_mm = int((hi[t] - lo[t]).sum())
                        assert n_mm > 0
                        mm = 0
                        for k in range(N_CHUNKS):
                            base = int(gsk_start[su][k])
                            for g in range(int(lo[t][k]), int(hi[t][k])):
                                if "mm" not in ablate or mm == 0:
                                    nc.tensor.matmul(
                                        acc[:], m2t[:, :, mm],
                                        gths[k][:, g - base, :],
                                        start=(mm == 0),
                                        stop=(mm == n_mm - 1
                                              or "mm" in ablate),
                                    )
                                mm += 1
                        emit_cb(t, u, acc, bt)
                    flush_cb(su, bt)

            # ---- phase P1: g1 = A^T g0, scaled by a^2
            def p1_flush(s, bt):
                nc.sync.dma_start(sh_rows(g1_sh[s // NSH2], s % NSH2), bt[:])
                if s % NSH2 == NSH2 - 1:
                    allgather(g1_sh[s // NSH2], g1_rep[s // NSH2])

            prop_step(
                g0_rep,
                lambda s: op.tile([128, TS, DIM], f16, tag="g1p", name="g1p"),
                lambda t, u, acc, bt: nc.scalar.mul(
                    bt[:, u, :], acc[:], a2_sb[:, t:t + 1]),
                p1_flush,
            )

            # ---- phase P2 + inverse rotation + FFN, fused per super-tile
            state = {}

            def p2_alloc(s):
                rp2 = iop.tile([128, TS, DIM], f16, tag="rp2")
                nc.sync.dma_start(rp2[:], sh_rows(rep_sh, s))
                h2s = rotp.tile([128, TS, DIM], f16, tag="h2s")
                state["rp2"] = rp2
                state["h2s"] = h2s
                return op.tile([128, TS, DIM], f32, tag="op2", name="op2")

            def p2_emit(t, u, acc, bt):
                # PSUM evict + a-scale into the super-tile h2 buffer
                nc.scalar.mul(state["h2s"][:, u, :], acc[:],
                              a_sb[:, t:t + 1])

            def p2_flush(s, bt):
                h2s, rp2 = state["h2s"], state["rp2"]
                h3 = rotp.tile([128, TS, DIM], f16, tag="h3")
                rotation_q(h2s[:], rp2[:], h3, "h3", transposed=True)
                # transpose each tile to [feat, node]
                h3t = rotp.tile([128, TS, 128], f16, tag="h3t")
                for u in range(TS):
                    tp = ppA.tile([128, 128], f16, tag="tp")
                    nc.tensor.transpose(tp[:], h3[:, u, :], id_sb[:])
                    nc.scalar.copy(h3t[:, u, :], tp[:])
                # FFN over node chunks of <=4 tiles (512 cols)
                act = rotp.tile([128, 2, TS, 128], f16, tag="act")
                for c0, cw in ((0, 4), (4, 3)):
                    w = cw * 128
                    h3c = h3t[:, c0:c0 + cw, :].rearrange("p q d -> p (q d)")
                    for h in range(2):
                        ps1 = ppB.tile([128, 512], f32, tag="ps1")
                        nc.tensor.matmul(ps1[:, 0:w],
                                         w1t_sb[:, h * 128:(h + 1) * 128],
                                         h3c, start=True, stop=True)
                        nc.scalar.activation(
                            act[:, h, c0:c0 + cw, :].rearrange(
                                "p q d -> p (q d)"),
                            ps1[:, 0:w],
                            mybir.ActivationFunctionType.Gelu,
                            bias=b1h_sb[:, h:h + 1])
                    ps2 = ppB.tile([128, 512], f32, tag="ps2")
                    for h in range(2):
                        nc.tensor.matmul(
                            ps2[:, 0:w], w2t2_sb[:, h, :],
                            act[:, h, c0:c0 + cw, :].rearrange(
                                "p q d -> p (q d)"),
                            start=(h == 0), stop=(h == 1))
                    nc.scalar.activation(
                        bt[:, c0:c0 + cw, :].rearrange("p q d -> p (q d)"),
                        ps2[:, 0:w],
                        mybir.ActivationFunctionType.Identity,
                        bias=b2c_sb[:])
                nc.sync.dma_start(
                    out_t[:, s * TS * 128:(s + 1) * TS * 128],
                    bt[:].rearrange("p q d -> p (q d)"))

            prop_step(g1_rep, p2_alloc, p2_emit, p2_flush)

    nc.compile()
    return nc


# -------------------------------------------------------------------- runner

def kernel(x, node_rep, src, dst, w1, b1, w2, b2):
    global LAST_RESULTS, LAST_NC, LAST_IN_MAPS
    from concourse import bass_utils

    x = np.asarray(x, np.float32)
    node_rep = np.asarray(node_rep, np.float32)
    src = np.asarray(src, np.int64)
    dst = np.asarray(dst, np.int64)
    w1 = np.asarray(w1, np.float32)
    b1 = np.asarray(b1, np.float32)
    w2 = np.asarray(w2, np.float32)
    b2 = np.asarray(b2, np.float32)

    in_maps, structure, total_groups = preprocess(
        x, node_rep, src, dst, w1, b1, w2, b2)
    nc = build_nc(structure, total_groups)
    res = bass_utils.run_bass_kernel_spmd(
        nc, in_maps, core_ids=list(range(N_CORES)),
        trace=bool(os.environ.get("BASS_TRACE")),
    )
    LAST_RESULTS = res
    LAST_NC = nc
    LAST_IN_MAPS = in_maps
    out = np.concatenate(
        [res.results[c]["out_t"].T[:NSH] for c in range(N_CORES)], axis=0)
    return np.ascontiguousarray(out)


# revision 14
# speedup vs baseline: 10.2603x; 3.5668x over previous
"""Trainium2 Bass kernel for GNN message passing (nn_BDLModule_34488587387542).

Computation (N=100000 nodes, E=1600000 edges, DIM=128):
    deg  = out-degree(src);  a = rsqrt(deg)
    h0   = per-node block rotation of x (8 bundles of 4x4)
    h2   = S S h0,  S = diag(a) A^T diag(a)   (2 propagation steps)
    h3   = inverse rotation of h2
    out  = GELU_exact(h3 @ w1.T + b1) @ w2.T + b2

Sharding: nodes partitioned contiguously across 8 cores (12500 each). Edges
bucketed by owning dst shard; per core, grouped by (dst tile of 128, src
chunk of 25088 replica rows) so gathers use int16 indices. The propagation
step is: dma_gather rows from the replicated node table (SWDGE on Pool —
the only work left on Pool), build one-hot dst matrices on DVE in one
batched is_equal per dst tile (tile-major packed dst stream), accumulate
with PE matmuls into PSUM. AllGather replicates the node table between
steps. The separable norm coefs a[src]*a[dst] are folded into the stored
node tables, never per-edge. Rotations run batched per super-tile on DVE
via the affine (q b)-merged view; FFN matmuls are batched across 4-tile
chunks of nodes. All DVE-path data is fp16 (fp32 accumulate in PSUM).
"""
import os
import sys

sys.path.append("/opt/trn_rl_repo")

import numpy as np

N_NODES = 100000
N_EDGES = 1600000
DIM = 128
HID = 256
N_CORES = 8
NSH = 12500                 # nodes per shard
NSHP = 12544                # padded shard rows (98 * 128)
NT = NSHP // 128            # dst tiles per core = 98
NREP = NSHP * N_CORES       # replica table rows = 100352
CHUNK = NREP // 4           # gather chunk rows = 25088 (int16-addressable)
N_CHUNKS = 4
PAD_DST = 1000.0            # dst_local sentinel for padding edges
TS = 7                      # dst tiles per super-tile (gathers merged per
NS = NT // TS               # (super-tile, chunk) to amortize SWDGE overhead)

# module globals: last run state (test.py reuses these for timing)
LAST_RESULTS = None
LAST_NC = None
LAST_IN_MAPS = None


# ----------------------------------------------------------------- host prep

def _wrap_idx(idx_flat: np.ndarray) -> np.ndarray:
    """[n] -> [128, n/16] int16 wrapped+replicated dma_gather index layout."""
    w = idx_flat.reshape(-1, 16).T.astype(np.int16)
    return np.tile(w, (8, 1))


def preprocess(x, node_rep, src, dst, w1, b1, w2, b2):
    """Build per-core input maps + the static SPMD edge-group structure."""
    deg = np.bincount(src, minlength=N_NODES).astype(np.float64)
    a64 = 1.0 / np.sqrt(deg)
    a = a64.astype(np.float32)
    a2 = (1.0 / deg).astype(np.float32)

    # global node id -> replica-table row. Layout is half-shard major:
    # half h (rows [h*6272, ...) of each shard) occupies replica rows
    # [h*50176 + shard*6272, ...+6272), so chunks 0-1 depend only on the
    # first-half AllGather and chunks 2-3 on the second.
    HALF = NSHP // 2

    def rrow(u):
        c = u // NSH
        r = u % NSH
        h = r // HALF
        return h * (N_CORES * HALF) + c * HALF + (r - h * HALF)

    src_rrow = rrow(src)
    dst_core = dst // NSH

    # Edge stream packed per (super-tile s, chunk k): the 7 tiles' edges
    # concatenated (tile order), padded to a multiple of 128 shared across
    # cores. Groups of 128 may straddle tile boundaries; a straddling group
    # is visited by both tiles with complementary PAD masks in their dst
    # streams, so the gather stream carries ~4% padding instead of ~25%.
    per_core = []
    cnt_tuk = np.zeros((N_CORES, NS, N_CHUNKS, TS), np.int64)
    for c in range(N_CORES):
        m = dst_core == c
        dl = (dst[m] - c * NSH).astype(np.int64)      # local dst
        sr = src_rrow[m]
        tile_id = dl // 128
        chunk_id = sr // CHUNK
        key = ((tile_id // TS) * N_CHUNKS + chunk_id) * TS + tile_id % TS
        order = np.argsort(key, kind="stable")
        per_core.append((dl[order], sr[order], key[order]))
        np.add.at(cnt_tuk, (c, tile_id // TS, chunk_id, tile_id % TS), 1)

    cnt_sk = cnt_tuk.sum(axis=3)                      # [C, NS, K]
    Gsk = np.ceil(cnt_sk.max(axis=0) / 128.0).astype(np.int64)   # [NS, K]
    gsk_start = np.concatenate(
        [[0], np.cumsum(Gsk.reshape(-1))]).reshape(-1)[:-1].reshape(NS,
                                                                    N_CHUNKS)
    total_groups = int(Gsk.sum())
    total_edges_padded = total_groups * 128

    # per-(core, s, k, u): start offset of tile u's edges in the (s,k) stream
    start_u = np.concatenate(
        [np.zeros((N_CORES, NS, N_CHUNKS, 1), np.int64),
         np.cumsum(cnt_tuk, axis=3)], axis=3)          # [C, NS, K, TS+1]
    # shared group ranges per (tile, chunk): groups the tile may touch on
    # any core
    lo = np.zeros((NT, N_CHUNKS), np.int64)
    hi = np.zeros((NT, N_CHUNKS), np.int64)
    for su in range(NS):
        for k in range(N_CHUNKS):
            for u in range(TS):
                t = su * TS + u
                s0 = start_u[:, su, k, u].min() // 128
                e1 = start_u[:, su, k, u + 1].max()
                e1 = -(-e1 // 128)
                if e1 <= s0:
                    s0 = e1 = 0
                lo[t, k] = gsk_start[su, k] + s0
                hi[t, k] = gsk_start[su, k] + e1
    gt_per_tile = (hi - lo).sum(axis=1)
    tm_off = np.concatenate([[0], np.cumsum(gt_per_tile)])
    total_tm = int(tm_off[-1])
    gt_max = int(gt_per_tile.max())
    structure = {"Gsk": Gsk, "gsk_start": gsk_start, "lo": lo, "hi": hi,
                 "tm_off": tm_off, "total_tm": total_tm}

    # iota3[p, c, g] = c: one-hot comparand with the group axis innermost so
    # every operand of the batched is_equal is stride-1 (DVE 2x mode)
    iota3 = np.ascontiguousarray(np.broadcast_to(
        np.arange(128, dtype=np.float16)[None, :, None], (128, 128, gt_max)))
    iota = np.tile(np.arange(128, dtype=np.float16), (128, 1))
    ident = np.eye(128, dtype=np.float16)
    w1t = np.ascontiguousarray(w1.T).astype(np.float16)        # [DIM, HID]
    b1h = np.ascontiguousarray(b1.reshape(2, 128).T)           # [128, 2]
    w2t2 = np.ascontiguousarray(
        w2.T.reshape(2, 128, DIM).transpose(1, 0, 2)).astype(np.float16)
    b2c = np.ascontiguousarray(b2.reshape(128, 1))

    in_maps = []
    for c in range(N_CORES):
        dl, sr, key = per_core[c]
        # slot position of each edge: groups packed per (s,k) bucket
        bucket_starts = (gsk_start.reshape(-1) * 128)[
            np.arange(NS * N_CHUNKS).reshape(NS, N_CHUNKS).repeat(TS).reshape(
                NS, N_CHUNKS, TS).reshape(-1)]        # [n_buckets] by key
        n_buckets = NS * N_CHUNKS * TS
        bcnt = np.bincount(key, minlength=n_buckets)
        within = np.arange(dl.shape[0]) - np.concatenate(
            [[0], np.cumsum(bcnt)])[key]
        # offset of tile-u block inside its (s,k) stream
        su_of = key // (N_CHUNKS * TS)
        k_of = (key // TS) % N_CHUNKS
        u_of = key % TS
        pos = (bucket_starts[key] + start_u[c, su_of, k_of, u_of] + within)
        idx_pad = np.zeros(total_edges_padded, np.int64)
        dst_pad = np.full(total_edges_padded, PAD_DST, np.float32)
        tile_pad = np.full(total_edges_padded, -1, np.int64)
        idx_pad[pos] = sr % CHUNK
        dst_pad[pos] = (dl % 128).astype(np.float32)
        tile_pad[pos] = dl // 128

        # tile-major masked dst stream: for tile t, chunk k, groups
        # [lo, hi): dst row where the slot's edge belongs to t, else PAD
        dst_tm = np.full((total_tm, 128), PAD_DST, np.float32)
        col = 0
        dg = dst_pad.reshape(total_groups, 128)
        tg = tile_pad.reshape(total_groups, 128)
        for t in range(NT):
            for k in range(N_CHUNKS):
                for g in range(int(lo[t, k]), int(hi[t, k])):
                    mask = tg[g] == t
                    dst_tm[col, mask] = dg[g, mask]
                    col += 1
        assert col == total_tm

        rows = slice(c * NSH, (c + 1) * NSH)
        x_sh = np.zeros((NSHP, DIM), np.float16)
        x_sh[:NSH] = x[rows].astype(np.float16)
        # pre-swizzled [NS, 128, TS*DIM] so per-super-tile loads are
        # contiguous per partition
        x_tiles = np.ascontiguousarray(
            x_sh.reshape(NS, TS, 128, DIM).transpose(0, 2, 1, 3).reshape(
                NS, 128, TS * DIM))
        # rotation operand split per contraction index d so every DVE
        # operand is stride-1 in its last dim (2x mode):
        # fwd: repf_d[n, (b,c,e)] = rep[n,b,c,d]; inv: repi_d = rep[n,b,d,c]
        rep4 = node_rep[rows].astype(np.float16)        # [NSH, 8, 4, 4]
        repf = np.zeros((4, NSHP, DIM), np.float16)
        repi = np.zeros((4, NSHP, DIM), np.float16)
        for d in range(4):
            repf[d, :NSH] = np.broadcast_to(
                rep4[:, :, :, d][:, :, :, None], (NSH, 8, 4, 4)
            ).reshape(NSH, DIM)
            repi[d, :NSH] = np.broadcast_to(
                rep4[:, :, d, :][:, :, :, None], (NSH, 8, 4, 4)
            ).reshape(NSH, DIM)

        def tiles4(r):
            return np.ascontiguousarray(
                r.reshape(4, NS, TS, 128, DIM).transpose(0, 1, 3, 2, 4)
                .reshape(4, NS, 128, TS * DIM))

        repf_t = tiles4(repf)
        repi_t = tiles4(repi)
        a_pad = np.zeros(NSHP, np.float32)
        a_pad[:NSH] = a[rows]
        a2_pad = np.zeros(NSHP, np.float32)
        a2_pad[:NSH] = a2[rows]

        in_maps.append({
            "x_sh": x_tiles,
            "repf": repf_t,
            "repi": repi_t,
            "idx_all": _wrap_idx(idx_pad),                       # [128, tg*8]
            "dst_all": np.ascontiguousarray(
                dst_tm.T.astype(np.float16)),                    # [128, ttm]
            "a_col": np.ascontiguousarray(
                a_pad.reshape(NT, 128).T),                       # [128, NT]
            "a2_col": np.ascontiguousarray(
                a2_pad.reshape(NT, 128).T),                      # [128, NT]
            "iota": iota,
            "iota3": iota3,
            "ident": ident,
            "w1t": w1t,
            "b1h": b1h,
            "w2t2": w2t2,
            "b2c": b2c,
        })
    return in_maps, structure, total_groups


# -------------------------------------------------------------- device build

def build_nc(structure, total_groups, single_core_timing=False, ablate=(),
             n_queues=4, gather_split=1):
    import concourse.bacc as bacc
    import concourse.mybir as mybir
    import concourse.tile as tile

    f32 = mybir.dt.float32
    f16 = mybir.dt.float16
    nc = bacc.Bacc("TRN2", target_bir_lowering=False, debug=False,
                   num_devices=1 if single_core_timing else N_CORES,
                   num_swdge_queues=n_queues)

    x_sh = nc.dram_tensor("x_sh", [NS, 128, TS * DIM], f16,
                          kind="ExternalInput")
    repf = nc.dram_tensor("repf", [4, NS, 128, TS * DIM], f16,
                          kind="ExternalInput")
    repi = nc.dram_tensor("repi", [4, NS, 128, TS * DIM], f16,
                          kind="ExternalInput")
    idx_all = nc.dram_tensor("idx_all", [128, total_groups * 8],
                             mybir.dt.int16, kind="ExternalInput")
    total_tm = structure["total_tm"]
    dst_all = nc.dram_tensor("dst_all", [128, total_tm], f16,
                             kind="ExternalInput")
    a_col = nc.dram_tensor("a_col", [128, NT], f32, kind="ExternalInput")
    a2_col = nc.dram_tensor("a2_col", [128, NT], f32, kind="ExternalInput")
    iota = nc.dram_tensor("iota", [128, 128], f16, kind="ExternalInput")
    ident = nc.dram_tensor("ident", [128, 128], f16, kind="ExternalInput")
    w1t = nc.dram_tensor("w1t", [DIM, HID], f16, kind="ExternalInput")
    b1h = nc.dram_tensor("b1h", [128, 2], f32, kind="ExternalInput")
    w2t2 = nc.dram_tensor("w2t2", [128, 2, DIM], f16, kind="ExternalInput")
    b2c = nc.dram_tensor("b2c", [128, 1], f32, kind="ExternalInput")
    out_t = nc.dram_tensor("out_t", [DIM, NSHP], f32, kind="ExternalOutput")

    Gsk = structure["Gsk"]
    gsk_start = structure["gsk_start"]
    lo = structure["lo"]
    hi = structure["hi"]
    tm_off = structure["tm_off"]
    gsk_max = int(Gsk.max())
    gt_max = int((hi - lo).sum(axis=1).max())
    iota3 = nc.dram_tensor("iota3", [128, 128, gt_max], f16,
                           kind="ExternalInput")
    with tile.TileContext(nc) as tc:
        with (
            tc.tile_pool(name="const", bufs=1) as cp,
            tc.tile_pool(name="io", bufs=2) as iop,
            tc.tile_pool(name="rotp", bufs=2) as rotp,
            tc.tile_pool(name="gath", bufs=8) as gp,
            tc.tile_pool(name="m2", bufs=4) as m2p,
            tc.tile_pool(name="outp", bufs=2) as op,
            tc.tile_pool(name="psA", bufs=3, space="PSUM") as ppA,
            tc.tile_pool(name="psB", bufs=1, space="PSUM") as ppB,
            tc.tile_pool(name="dram", bufs=1, space="DRAM") as dp,
        ):
            # ---- constants into SBUF
            iota_sb = cp.tile([128, 128], f16)
            nc.sync.dma_start(iota_sb[:], iota[:])
            iota3_sb = cp.tile([128, 128, gt_max], f16)
            nc.sync.dma_start(iota3_sb[:], iota3[:])
            id_sb = cp.tile([128, 128], f16)
            nc.sync.dma_start(id_sb[:], ident[:])
            idx_sb = cp.tile([128, total_groups * 8], mybir.dt.int16)
            nc.sync.dma_start(idx_sb[:], idx_all[:])
            dst_sb = cp.tile([128, total_tm], f16)
            nc.sync.dma_start(dst_sb[:], dst_all[:])
            a_sb = cp.tile([128, NT], f32)
            nc.sync.dma_start(a_sb[:], a_col[:])
            a2_sb = cp.tile([128, NT], f32)
            nc.sync.dma_start(a2_sb[:], a2_col[:])
            w1t_sb = cp.tile([DIM, HID], f16)
            nc.sync.dma_start(w1t_sb[:], w1t[:])
            b1h_sb = cp.tile([128, 2], f32)
            nc.sync.dma_start(b1h_sb[:], b1h[:])
            w2t2_sb = cp.tile([128, 2, DIM], f16)
            nc.sync.dma_start(w2t2_sb[:], w2t2[:])
            b2c_sb = cp.tile([128, 1], f32)
            nc.sync.dma_start(b2c_sb[:], b2c[:])

            rep_space = "Local" if single_core_timing else "Shared"
            HALF = NSHP // 2
            NSH2 = NS // 2
            g0_sh = [dp.tile([HALF, DIM], f16, name=f"g0sh{h}")
                     for h in range(2)]
            g0_rep = [dp.tile([HALF * N_CORES, DIM], f16,
                              addr_space=rep_space, name=f"g0rep{h}")
                      for h in range(2)]
            g1_sh = [dp.tile([HALF, DIM], f16, name=f"g1sh{h}")
                     for h in range(2)]
            g1_rep = [dp.tile([HALF * N_CORES, DIM], f16,
                              addr_space=rep_space, name=f"g1rep{h}")
                      for h in range(2)]

            def allgather(sh, rep):
                # one half-shard AllGather; fires as soon as its half is
                # written, overlapping the other half's compute
                if single_core_timing:
                    # timing stand-in: local copy keeps the data dependency
                    nc.sync.dma_start(rep[0:HALF, :], sh[:])
                else:
                    nc.gpsimd.collective_compute(
                        "AllGather", mybir.AluOpType.bypass,
                        ins=[sh.opt()], outs=[rep.opt()],
                        replica_groups=[list(range(N_CORES))],
                    )

            def rotation_q(x_ap, repd_ap, dest, dest_tag, q=TS):
                """dest[:, q, :] = per-node block rotation, batched over q
                tiles. The (q, bundle) axes merge into one affine dim m
                (q-stride 128 = 8 x bundle-stride 16). repd_ap holds the
                4 per-d expanded planes [128, q, 4, DIM] so every operand
                is packed stride-1 in its last dim (DVE 2x mode)."""
                m = q * 8
                if "rot" in ablate:  # timing placeholder
                    nc.vector.tensor_copy(dest[:], x_ap)
                    return
                x4 = x_ap.rearrange("p q (b d e) -> p (q b) d e", b=8, d=4,
                                    e=4)
                tmp = rotp.tile([128, q, DIM], f16, tag=dest_tag + "_tmp")
                for d in range(4):
                    a_d = repd_ap[:, d, :, :].rearrange(
                        "p q (b c e) -> p (q b) c e", b=8, c=4, e=4)
                    b_d = x4[:, :, d, :].unsqueeze(2).broadcast_to(
                        (128, m, 4, 4))
                    dst4 = (dest if d == 0 else tmp)[:].rearrange(
                        "p q (b c e) -> p (q b) c e", b=8, c=4, e=4)
                    nc.vector.tensor_tensor(dst4, a_d, b_d,
                                            op=mybir.AluOpType.mult)
                    if d > 0:
                        nc.vector.tensor_tensor(dest[:], dest[:], tmp[:],
                                                op=mybir.AluOpType.add)

            def sh_rows(dram, s):
                """[128, TS, DIM] view of a shard's super-tile s rows."""
                return dram[s * TS * 128:(s + 1) * TS * 128, :].rearrange(
                    "(q p) d -> p q d", p=128)


            # ---- phase R1: g0 = rotate(x) * a
            for s in range(NS):
                xp = iop.tile([128, TS, DIM], f16, tag="xp")
                nc.sync.dma_start(
                    xp[:].rearrange("p q d -> p (q d)"), x_sh[s, :, :])
                rp = iop.tile([128, 4, TS, DIM], f16, tag="rp")
                for d in range(4):
                    nc.sync.dma_start(
                        rp[:, d, :, :].rearrange("p q d -> p (q d)"),
                        repf[d, s, :, :])
                rot = rotp.tile([128, TS, DIM], f16, tag="rot")
                rotation_q(xp[:], rp[:], rot, "rot")
                g0p = op.tile([128, TS, DIM], f16, tag="g0p")
                for u in range(TS):
                    t = s * TS + u
                    nc.scalar.mul(g0p[:, u, :], rot[:, u, :],
                                  a_sb[:, t:t + 1])
                nc.sync.dma_start(sh_rows(g0_sh[s // NSH2], s % NSH2),
                                  g0p[:])
                if s % NSH2 == NSH2 - 1:
                    allgather(g0_sh[s // NSH2], g0_rep[s // NSH2])

            def onehots(t, tag):
                """[128, 128, gt] one-hot stack for dst tile t (one DVE op;
                group axis innermost so all operands are stride-1 -> 2x)."""
                gt = int(tm_off[t + 1] - tm_off[t])
                o0 = int(tm_off[t])
                m2t = m2p.tile([128, 128, gt_max], f16, tag=tag)
                if "onehot" not in ablate:
                    nc.vector.tensor_tensor(
                        m2t[:, :, 0:gt],
                        iota3_sb[:, :, 0:gt],
                        dst_sb[:, o0:o0 + gt].unsqueeze(1).broadcast_to(
                            (128, 128, gt)),
                        op=mybir.AluOpType.is_equal)
                else:  # timing placeholder: tiny op
                    nc.vector.tensor_scalar(
                        m2t[:, 0:1, 0:8], iota_sb[:, 0:8], 0.0,
                        None, op0=mybir.AluOpType.mult)
                return m2t

            def prop_step(g_rep, alloc_cb, emit_cb, flush_cb):
                """One propagation step. One merged gather per (super-tile,
                chunk); per dst tile, a batched one-hot build then one-hot
                matmuls accumulating in PSUM. Outputs are batched per
                super-tile via the callbacks."""
                for su in range(NS):
                    gths = {}
                    for k in range(N_CHUNKS):
                        gc = int(Gsk[su][k])
                        if gc == 0:
                            continue
                        c0 = int(gsk_start[su][k])
                        gth = gp.tile([128, gsk_max, DIM], f16, tag="gth")
                        if "gather" not in ablate:
                            bounds = [gc * i // gather_split
                                      for i in range(gather_split + 1)]
                            for h in range(gather_split):
                                glo, ghi = bounds[h], bounds[h + 1]
                                if glo == ghi:
                                    continue
                                nc.gpsimd.dma_gather(
                                    gth[:, glo:ghi, :],
                                    g_rep[k // 2][
                                        (k % 2) * CHUNK:
                                        (k % 2 + 1) * CHUNK, :],
                                    idx_sb[:, (c0 + glo) * 8:(c0 + ghi) * 8],
                                    128 * (ghi - glo), 128 * (ghi - glo),
                                    DIM,
                                    single_packet=False,
                                    queue_num=(gather_split * k + h)
                                    % n_queues,
                                )
                        else:  # timing placeholder: cheap small copy
                            nc.sync.dma_start(gth[:, 0, :],
                                              g_rep[0][0:128, :])
                        gths[k] = gth
                    bt = alloc_cb(su)
                    for u in range(TS):
                        t = su * TS + u
                        m2t = onehots(t, "m2t")
                        acc = ppA.tile([128, DIM], f32, tag="acc")
                        n_mm = int((hi[t] - lo[t]).sum())
                        assert n_mm > 0
                        mm = 0
                        for k in range(N_CHUNKS):
                            base = int(gsk_start[su][k])
                            for g in range(int(lo[t][k]), int(hi[t][k])):
                                if "mm" not in ablate or mm == 0:
                                    nc.tensor.matmul(
                                        acc[:], m2t[:, :, mm],
                                        gths[k][:, g - base, :],
                                        start=(mm == 0),
                                        stop=(mm == n_mm - 1
                                              or "mm" in ablate),
                                    )
                                mm += 1
                        emit_cb(t, u, acc, bt)
                    flush_cb(su, bt)

            # ---- phase P1: g1 = A^T g0, scaled by a^2
            def p1_flush(s, bt):
                nc.sync.dma_start(sh_rows(g1_sh[s // NSH2], s % NSH2), bt[:])
                if s % NSH2 == NSH2 - 1:
                    allgather(g1_sh[s // NSH2], g1_rep[s // NSH2])

            prop_step(
                g0_rep,
                lambda s: op.tile([128, TS, DIM], f16, tag="g1p", name="g1p"),
                lambda t, u, acc, bt: nc.scalar.mul(
                    bt[:, u, :], acc[:], a2_sb[:, t:t + 1]),
                p1_flush,
            )

            # ---- phase P2 + inverse rotation + FFN, fused per super-tile
            state = {}

            def p2_alloc(s):
                rp2 = iop.tile([128, 4, TS, DIM], f16, tag="rp2")
                for d in range(4):
                    nc.sync.dma_start(
                        rp2[:, d, :, :].rearrange("p q d -> p (q d)"),
                        repi[d, s, :, :])
                h2s = rotp.tile([128, TS, DIM], f16, tag="h2s")
                state["rp2"] = rp2
                state["h2s"] = h2s
                return op.tile([128, TS, DIM], f32, tag="op2", name="op2")

            def p2_emit(t, u, acc, bt):
                # PSUM evict + a-scale into the super-tile h2 buffer
                nc.scalar.mul(state["h2s"][:, u, :], acc[:],
                              a_sb[:, t:t + 1])

            def p2_flush(s, bt):
                h2s, rp2 = state["h2s"], state["rp2"]
                h3 = rotp.tile([128, TS, DIM], f16, tag="h3")
                rotation_q(h2s[:], rp2[:], h3, "h3")
                # transpose each tile to [feat, node]
                h3t = rotp.tile([128, TS, 128], f16, tag="h3t")
                for u in range(TS):
                    tp = ppA.tile([128, 128], f16, tag="tp")
                    nc.tensor.transpose(tp[:], h3[:, u, :], id_sb[:])
                    nc.scalar.copy(h3t[:, u, :], tp[:])
                # FFN over node chunks of <=4 tiles (512 cols)
                act = rotp.tile([128, 2, TS, 128], f16, tag="act")
                for c0, cw in ((0, 4), (4, 3)):
                    w = cw * 128
                    h3c = h3t[:, c0:c0 + cw, :].rearrange("p q d -> p (q d)")
                    for h in range(2):
                        ps1 = ppB.tile([128, 512], f32, tag="ps1")
                        nc.tensor.matmul(ps1[:, 0:w],
                                         w1t_sb[:, h * 128:(h + 1) * 128],
                                         h3c, start=True, stop=True)
                        nc.scalar.activation(
                            act[:, h, c0:c0 + cw, :].rearrange(
                                "p q d -> p (q d)"),
                            ps1[:, 0:w],
                            mybir.ActivationFunctionType.Gelu,
                            bias=b1h_sb[:, h:h + 1])
                    ps2 = ppB.tile([128, 512], f32, tag="ps2")
                    for h in range(2):
                        nc.tensor.matmul(
                            ps2[:, 0:w], w2t2_sb[:, h, :],
                            act[:, h, c0:c0 + cw, :].rearrange(
                                "p q d -> p (q d)"),
                            start=(h == 0), stop=(h == 1))
                    nc.scalar.activation(
                        bt[:, c0:c0 + cw, :].rearrange("p q d -> p (q d)"),
                        ps2[:, 0:w],
                        mybir.ActivationFunctionType.Identity,
                        bias=b2c_sb[:])
                nc.sync.dma_start(
                    out_t[:, s * TS * 128:(s + 1) * TS * 128],
                    bt[:].rearrange("p q d -> p (q d)"))

            prop_step(g1_rep, p2_alloc, p2_emit, p2_flush)

    nc.compile()
    return nc


# -------------------------------------------------------------------- runner

def kernel(x, node_rep, src, dst, w1, b1, w2, b2):
    global LAST_RESULTS, LAST_NC, LAST_IN_MAPS
    from concourse import bass_utils

    x = np.asarray(x, np.float32)
    node_rep = np.asarray(node_rep, np.float32)
    src = np.asarray(src, np.int64)
    dst = np.asarray(dst, np.int64)
    w1 = np.asarray(w1, np.float32)
    b1 = np.asarray(b1, np.float32)
    w2 = np.asarray(w2, np.float32)
    b2 = np.asarray(b2, np.float32)

    in_maps, structure, total_groups = preprocess(
        x, node_rep, src, dst, w1, b1, w2, b2)
    nc = build_nc(structure, total_groups)
    res = bass_utils.run_bass_kernel_spmd(
        nc, in_maps, core_ids=list(range(N_CORES)),
        trace=bool(os.environ.get("BASS_TRACE")),
    )
    LAST_RESULTS = res
    LAST_NC = nc
    LAST_IN_MAPS = in_maps
    out = np.concatenate(
        [res.results[c]["out_t"].T[:NSH] for c in range(N_CORES)], axis=0)
    return np.ascontiguousarray(out)
